# revision 11
# baseline (speedup 1.0000x reference)
"""Gated MSA-row attention (AlphaFold-style) Trainium2 kernel.

Sharding: data-parallel over the 128 MSA rows -> 16 rows/core on 8 cores;
rows processed in pairs, software-pipelined (back of pair p-1 interleaved
with front of pair p so PE/ACT/DVE all stay busy and the PE never drops
out of its high p-state).

- transposed activations (qT/kT [hc, (row,pos)]) -> no on-chip transposes
- bias2 is ADDED into the logits PSUM pre-exp via identity-stationary
  matmuls (one per 2KB PSUM region, so start/accumulate flags line up
  with the zero-region tracker) -> the post-exp eb2 hadamard (the single
  biggest DVE cost) disappears; ACT's exp output IS a' directly
- logits layout keeps the baseline hp-major columns (hp*512 + kc*256) so
  the 4 concurrently-running row-tiled qk strips land in 4 DISTINCT PSUM
  banks (same-bank concurrent drains fault the engine)
- bias1 = exp(b1-2) rides on v (fused into the required PSUM->fp16 evac)
  and on the denominator weights (with the sigmoid 1/2 folded in)
- denominators via weighted ones-matmul over partitions, emerging
  pre-broadcast in per-head 32-strips; reciprocal_approx_fast
- gate uses tanh (same ACT table set as exp -> zero table switches):
  sigmoid(x) = (1 + tanh(x/2))/2 via scalar_tensor_tensor in the gating
- fp16 output evac split DVE/ACT, fp16 DMA out (host upcasts)
"""

import math
import sys

sys.path.insert(0, "/opt/trn_rl_repo")

import numpy as np

import concourse.bass as bass
import concourse.mybir as mybir
from concourse import bacc
from concourse.tile import TileContext

F32 = mybir.dt.float32
F16 = mybir.dt.float16

H = 8
CH = 32
CQ = 256
Q = 256
K = 256
S = 128
NCORES = 8
RPC = S // NCORES
PAIRS = RPC // 2

Exp = mybir.ActivationFunctionType.Exp
Tanh = mybir.ActivationFunctionType.Tanh
MULT = mybir.AluOpType.mult
ADD = mybir.AluOpType.add


def f_dma(nc, P, p):
    """Prefetch kv/q activations for pair p."""
    kv = [P["kvx"].tile([128, 512], F16, tag=f"kv{c}", name=f"kv{c}")
          for c in range(2)]
    qx = [P["qxp"].tile([128, 512], F16, tag=f"qx{c}", name=f"qx{c}")
          for c in range(2)]
    for c in range(2):
        nc.gpsimd.dma_start(kv[c][:], P["kvx_d"][p, c * 128:(c + 1) * 128, :])
        nc.gpsimd.dma_start(qx[c][:], P["qx_d"][p, c * 128:(c + 1) * 128, :])
    return {"kv": kv, "qx": qx}


def f_proj(nc, P, st, w, dst_pool, tag, m):
    """One 128-wide hc chunk of a kT/qT-style projection + fp16 evac."""
    src = st["kv"] if w == "wk" else st["qx"]
    msl = slice(m * 128, (m + 1) * 128)
    pk = P["pskv"].tile([128, 512], F32, tag="kv", name="pkv")
    nc.tensor.matmul(pk[:], P[f"{w}_sb"][0][:, msl], src[0][:],
                     start=True, stop=False)
    nc.tensor.matmul(pk[:], P[f"{w}_sb"][1][:, msl], src[1][:],
                     start=False, stop=True)
    t = dst_pool.tile([128, 512], F16, tag=f"{tag}{m}", name=f"{tag}{m}")
    nc.vector.tensor_copy(t[:], pk[:])
    return t


def f_kt(nc, P, st):
    with nc.named_scope("ktproj"):
        st["kt"] = [f_proj(nc, P, st, "wk", P["kt"], "kt", m)
                    for m in range(2)]


def f_qt(nc, P, st):
    with nc.named_scope("qproj"):
        st["qt"] = [f_proj(nc, P, st, "wq", P["qt"], "qt", m)
                    for m in range(2)]


def f_v(nc, P, st, p):
    """v projection -> fp16 [k-chunk, (kc,hc)] per row, scaled by eb1."""
    kv = st["kv"]
    v_sb = []
    for r in range(2):
        row = p * 2 + r
        with nc.named_scope("vproj"):
            pv = P["pskv"].tile([128, 512], F32, tag="kv", name="pkv")
            for kc in range(2):
                for c in range(2):
                    nc.tensor.matmul(
                        pv[:, kc * 256:(kc + 1) * 256],
                        kv[c][:, r * 256 + kc * 128:r * 256 + kc * 128 + 128],
                        P["wv_sb"][c][:],
                        start=(c == 0), stop=(c == 1))
            vt = P["vt"].tile([128, 512], F16, tag=f"v{r}", name=f"v{r}")
            for kc in range(2):
                nc.vector.tensor_scalar_mul(
                    vt[:, kc * 256:(kc + 1) * 256],
                    pv[:, kc * 256:(kc + 1) * 256],
                    P["eb1_sb"][:, row * 2 + kc:row * 2 + kc + 1])
            v_sb.append(vt)
    st["v"] = v_sb


def f_g(nc, P, st):
    """gate projection -> tanh(x/2) on ACT."""
    qx = st["qx"]
    gt_sb = []
    for m in range(2):
        msl = slice(m * 128, (m + 1) * 128)
        with nc.named_scope("gproj"):
            pg = P["pskv"].tile([128, 512], F32, tag="kv", name="pkv")
            nc.tensor.matmul(pg[:], P["wg_sb"][0][:, msl], qx[0][:],
                             start=True, stop=False)
            nc.tensor.matmul(pg[:], P["wg_sb"][1][:, msl], qx[1][:],
                             start=False, stop=True)
            gt = P["gt"].tile([128, 512], F32, tag=f"gt{m}", name=f"gt{m}")
            nc.scalar.activation(gt[:], pg[:], Tanh, scale=0.5)
            gt_sb.append(gt)
    st["g"] = gt_sb


def f_lg(nc, P, p, st, i):
    """Logits group i = (r, hg): per-head-region bias2 pre-adds (identity
    matmuls, start=True on exactly one 2KB zero region each) + 8 qk strip
    matmuls in the baseline bank-spread layout -> one [128,2048] exp on
    ACT straight into a'."""
    r, hg = i >> 1, i & 1
    if "a" not in st:
        st["a"] = P["apool"].tile([128, 8192], F16, tag="a", name="a")
    with nc.named_scope("logits"):
        lt = P["pslt"].tile([128, 2048], F32, tag="lt", name="lt")
        for hp in range(4):
            nc.tensor.matmul(
                lt[:, hp * 512:hp * 512 + 512], P["id_sb"][:],
                P["b2_sb"][:, hg * 2048 + hp * 512:hg * 2048 + hp * 512 + 512],
                start=True, stop=False)
        for kc in range(2):
            for hp in range(4):
                stp = 32 * hp
                nc.tensor.matmul(
                    lt[:, hp * 512 + kc * 256:hp * 512 + kc * 256 + 256],
                    st["kt"][hg][stp:stp + 32,
                                 r * 256 + kc * 128:r * 256 + kc * 128 + 128],
                    st["qt"][hg][stp:stp + 32, r * 256:r * 256 + 256],
                    start=False, stop=(kc == 1),
                    tile_position=(stp, 0))
    with nc.named_scope("exp"):
        blk = hg * 2 + r
        av = st["a"][:].rearrange("p (kc blk rest) -> p kc blk rest",
                                  kc=2, blk=4)
        eo = av[:, :, blk, :].rearrange("p kc (hp q) -> p hp kc q", hp=4)
        nc.scalar.activation(eo, lt[:], Exp, bias=P["shift_sb"][:])


def b_dn(nc, P, st, p, r):
    """Denominators for row r: eb1-weighted ones-matmul over partitions,
    emerging pre-broadcast in per-head 32-strips; fast reciprocal."""
    row = p * 2 + r
    av = st["a"][:].rearrange("p (kc hg rr rest) -> p kc hg rr rest",
                              kc=2, hg=2, rr=2)
    bc = P["psbk"].tile([128, 512], F32, tag="bk", name="bc")
    with nc.named_scope("denom"):
        for hp in range(4):
            for kc in range(2):
                rhs = av[:, kc, :, r, hp * 256:hp * 256 + 256]
                w1 = (row * 2 + kc) * 32
                nc.tensor.matmul(
                    bc[32 * hp:32 * hp + 32, :],
                    P["eb1w_sb"][:, w1:w1 + 32], rhs,
                    start=(kc == 0), stop=(kc == 1),
                    tile_position=(0, 32 * hp))
    rc = P["rcp"].tile([128, 512], F32, tag=f"rc{r}", name=f"rc{r}")
    with nc.named_scope("recip"):
        nc.vector.reciprocal_approx_fast(rc[:], bc[:])
    st[f"rc{r}"] = rc


def b_av(nc, P, st, r):
    """AV for row r + gating: og = (1 + tanh) * oT * rc."""
    ot = P["psbk"].tile([128, 512], F32, tag="bk", name="ot")
    with nc.named_scope("av"):
        for hg in range(2):
            for hp in range(4):
                for kc in range(2):
                    off = kc * 4096 + (hg * 2 + r) * 1024 + hp * 256
                    nc.tensor.matmul(
                        ot[32 * hp:32 * hp + 32, hg * 256:hg * 256 + 256],
                        st["v"][r][:, kc * 256 + hg * 128 + 32 * hp:
                                   kc * 256 + hg * 128 + 32 * hp + 32],
                        st["a"][:, off:off + 256],
                        start=(kc == 0), stop=(kc == 1),
                        tile_position=(0, 32 * hp))
    og = []
    rc = st[f"rc{r}"]
    for hg in range(2):
        with nc.named_scope("gating"):
            csl = slice(hg * 256, hg * 256 + 256)
            tmp = P["gtmp"].tile([128, 256], F32, tag="gtmp", name="gtmp")
            nc.vector.scalar_tensor_tensor(
                tmp[:], st["g"][hg][:, r * 256:r * 256 + 256], 1.0,
                ot[:, csl], ADD, MULT)
            ogt = P["otg"].tile([128, 256], F16, tag=f"og{hg}",
                                name=f"og{hg}")
            nc.vector.tensor_tensor(ogt[:], tmp[:], rc[:, csl], MULT)
            og.append(ogt)
    st[f"og{r}"] = og


def b_out(nc, P, st, p, r):
    """Output projection for row r; fp16 evac (DVE/ACT alternating) + DMA."""
    row = p * 2 + r
    og = st[f"og{r}"]
    fin = P["psbk"].tile([128, 512], F32, tag="bk", name="fin")
    with nc.named_scope("outproj"):
        for qc in range(2):
            for hg in range(2):
                nc.tensor.matmul(
                    fin[:, qc * 256:qc * 256 + 256],
                    og[hg][:, qc * 128:qc * 128 + 128],
                    P["wo_sb"][hg][:],
                    start=(hg == 0), stop=(hg == 1))
    ob = P["osb"].tile([128, 512], F16, tag=f"ob{r}", name=f"ob{r}")
    with nc.named_scope("outevac"):
        if r == 0:
            nc.vector.tensor_copy(ob[:], fin[:])
        else:
            nc.scalar.copy(ob[:], fin[:])
    nc.sync.dma_start(
        P["out_d"][row].rearrange("(qc p) d -> p qc d", qc=2),
        ob[:].rearrange("p (qc d) -> p qc d", qc=2))


def build_nc():
    nc = bacc.Bacc("TRN2", target_bir_lowering=False)

    P = {}
    P["qx_d"] = nc.dram_tensor("qx", [PAIRS, CQ, 512], F16,
                               kind="ExternalInput")
    P["kvx_d"] = nc.dram_tensor("kvx", [PAIRS, CQ, 512], F16,
                                kind="ExternalInput")
    wd = {nm: nc.dram_tensor(f"w{nm}t", [CQ, 256], F16, kind="ExternalInput")
          for nm in ("q", "k", "v", "g", "o")}
    b1_d = nc.dram_tensor("eb1s", [128, 2 * RPC], F32, kind="ExternalInput")
    b1w_d = nc.dram_tensor("eb1w", [128, 2 * RPC * 32], F16,
                           kind="ExternalInput")
    b2_d = nc.dram_tensor("b2sb", [128, 4096], F16, kind="ExternalInput")
    id_d = nc.dram_tensor("ident", [128, 128], F16, kind="ExternalInput")
    P["out_d"] = nc.dram_tensor("out", [RPC, Q, 256], F16,
                                kind="ExternalOutput")

    with TileContext(nc) as tc:
        with (
            tc.tile_pool(name="const", bufs=1) as cpool,
            tc.tile_pool(name="kvx", bufs=2) as kv_pool,
            tc.tile_pool(name="qxp", bufs=2) as qx_pool,
            tc.tile_pool(name="kt", bufs=2) as kt_pool,
            tc.tile_pool(name="qt", bufs=2) as qt_pool,
            tc.tile_pool(name="gt", bufs=2) as gt_pool,
            tc.tile_pool(name="vt", bufs=2) as vt_pool,
            tc.tile_pool(name="apool", bufs=2) as a_pool,
            tc.tile_pool(name="rcp", bufs=2) as r_pool,
            tc.tile_pool(name="otg", bufs=2) as og_pool,
            tc.tile_pool(name="gtmp", bufs=2) as gtmp_pool,
            tc.tile_pool(name="osb", bufs=2) as o_pool,
            tc.tile_pool(name="pslt", bufs=1, space="PSUM") as ps_lt,
            tc.tile_pool(name="pskv", bufs=2, space="PSUM") as ps_kv,
            tc.tile_pool(name="psbk", bufs=2, space="PSUM") as ps_bk,
        ):
            for nm in ("q", "k", "v", "g", "o"):
                tiles = [cpool.tile([128, 256], F16, tag=f"w{nm}{c}",
                                    name=f"w{nm}{c}") for c in range(2)]
                for c in range(2):
                    nc.sync.dma_start(tiles[c][:],
                                      wd[nm][c * 128:(c + 1) * 128, :])
                P[f"w{nm}_sb"] = tiles
            eb1_sb = cpool.tile([128, 2 * RPC], F32, tag="eb1", name="eb1")
            nc.sync.dma_start(eb1_sb[:], b1_d[:])
            P["eb1_sb"] = eb1_sb
            eb1w_sb = cpool.tile([128, 2 * RPC * 32], F16, tag="eb1w",
                                 name="eb1w")
            nc.sync.dma_start(eb1w_sb[:], b1w_d[:])
            P["eb1w_sb"] = eb1w_sb
            b2_sb = cpool.tile([128, 4096], F16, tag="b2", name="b2")
            nc.sync.dma_start(b2_sb[:], b2_d[:])
            P["b2_sb"] = b2_sb
            id_sb = cpool.tile([128, 128], F16, tag="id", name="id")
            nc.sync.dma_start(id_sb[:], id_d[:])
            P["id_sb"] = id_sb
            shift_sb = cpool.tile([128, 1], F32, tag="shift", name="shift")
            nc.vector.memset(shift_sb[:], -4.0)
            P["shift_sb"] = shift_sb

            P.update({"kvx": kv_pool, "qxp": qx_pool, "kt": kt_pool,
                      "qt": qt_pool, "gt": gt_pool, "vt": vt_pool,
                      "apool": a_pool, "rcp": r_pool, "otg": og_pool,
                      "gtmp": gtmp_pool, "osb": o_pool, "pslt": ps_lt,
                      "pskv": ps_kv, "psbk": ps_bk})

            # software pipeline: iter p emits front(p) interleaved with
            # back(p-1); lg groups spaced ~2us apart so exp (ACT) keeps up
            # with the single-buffered lt PSUM tile.
            states = [None] * PAIRS
            states[0] = f_dma(nc, P, 0)
            for p in range(PAIRS + 1):
                st = states[p] if p < PAIRS else None
                bs = states[p - 1] if p >= 1 else None
                if st is not None:
                    if p + 1 < PAIRS:
                        states[p + 1] = f_dma(nc, P, p + 1)
                    f_kt(nc, P, st)
                    f_qt(nc, P, st)
                    f_lg(nc, P, p, st, 0)
                    if bs is not None:
                        b_dn(nc, P, bs, p - 1, 0)
                        b_av(nc, P, bs, 0)
                        b_out(nc, P, bs, p - 1, 0)
                    f_lg(nc, P, p, st, 1)
                    f_v(nc, P, st, p)
                    if bs is not None:
                        b_dn(nc, P, bs, p - 1, 1)
                    f_g(nc, P, st)
                    f_lg(nc, P, p, st, 2)
                    if bs is not None:
                        b_av(nc, P, bs, 1)
                        b_out(nc, P, bs, p - 1, 1)
                    f_lg(nc, P, p, st, 3)
                else:
                    b_dn(nc, P, bs, p - 1, 0)
                    b_av(nc, P, bs, 0)
                    b_out(nc, P, bs, p - 1, 0)
                    b_dn(nc, P, bs, p - 1, 1)
                    b_av(nc, P, bs, 1)
                    b_out(nc, P, bs, p - 1, 1)

    nc.compile()
    return nc


def host_prep(q_x, kv_x, bias1, bias2, wq, wk, wv, wg, wo):
    wqt = np.ascontiguousarray((wq / math.sqrt(CH)).T.astype(np.float16))
    wkt = np.ascontiguousarray(wk.T.astype(np.float16))
    wvt = np.ascontiguousarray(wv.T.astype(np.float16))
    wgt = np.ascontiguousarray(wg.T.astype(np.float16))
    wot = np.ascontiguousarray(wo.T.astype(np.float16))

    # b2sb[p, (hg, hp, kc, q)] = bias2[hg*4+hp, q, kc*128+p]
    b2 = bias2[0, 0].astype(np.float32)                     # [H, Q, K]
    t = b2.reshape(2, 4, Q, 2, 128)                         # hg hp q kc p
    b2sb = np.ascontiguousarray(
        t.transpose(4, 0, 1, 3, 2).reshape(128, 4096).astype(np.float16))

    ident = np.ascontiguousarray(np.eye(128, dtype=np.float16))

    in_maps = []
    for c in range(NCORES):
        rows = slice(c * RPC, (c + 1) * RPC)
        qx = q_x[0, rows]
        qxp = qx.reshape(PAIRS, 2, Q, CQ).transpose(0, 3, 1, 2)
        qxp = np.ascontiguousarray(qxp.reshape(PAIRS, CQ, 512)
                                   .astype(np.float16))
        kvx = kv_x[0, rows]
        kvp = kvx.reshape(PAIRS, 2, K, CQ).transpose(0, 3, 1, 2)
        kvp = np.ascontiguousarray(kvp.reshape(PAIRS, CQ, 512)
                                   .astype(np.float16))
        b1 = np.exp(bias1[0, rows, 0, 0, :].astype(np.float32) - 2.0)
        eb1s = np.ascontiguousarray(
            b1.reshape(RPC, 2, 128).transpose(2, 0, 1).reshape(128, 2 * RPC))
        # denominator weights also carry the sigmoid 1/2 (gating computes
        # (1+tanh) * oT * rc = 2*sigmoid * oT * rc)
        eb1w = np.ascontiguousarray(
            np.repeat((eb1s * 2.0)[:, :, None], 32, axis=2)
            .reshape(128, 2 * RPC * 32).astype(np.float16))
        in_maps.append({
            "qx": qxp, "kvx": kvp, "wqt": wqt, "wkt": wkt, "wvt": wvt,
            "wgt": wgt, "wot": wot, "eb1s": eb1s, "eb1w": eb1w,
            "b2sb": b2sb, "ident": ident,
        })
    return in_maps


def gather(results):
    out = np.empty((1, S, Q, CQ), dtype=np.float32)
    for c in range(NCORES):
        out[0, c * RPC:(c + 1) * RPC] = results[c]["out"].astype(np.float32)
    return out


_NC_CACHE = None


def kernel_traced(q_x, kv_x, bias1, bias2, wq, wk, wv, wg, wo, trace=False):
    """Returns (full output [1,128,256,256] fp32, BassKernelResults)."""
    from concourse.bass_utils import run_bass_kernel_spmd
    global _NC_CACHE
    if _NC_CACHE is None:
        _NC_CACHE = build_nc()
    q_x, kv_x = np.asarray(q_x), np.asarray(kv_x)
    bias1, bias2 = np.asarray(bias1), np.asarray(bias2)
    wq, wk, wv, wg, wo = (np.asarray(w) for w in (wq, wk, wv, wg, wo))
    in_maps = host_prep(q_x, kv_x, bias1, bias2, wq, wk, wv, wg, wo)
    res = run_bass_kernel_spmd(_NC_CACHE, in_maps, list(range(NCORES)),
                               trace=trace)
    return gather(res.results), res


def kernel(q_x, kv_x, bias1, bias2, wq, wk, wv, wg, wo):
    """Full (unsharded) inputs in, full output out. Shards the 128 MSA
    rows across the 8 NeuronCores internally."""
    out, _ = kernel_traced(q_x, kv_x, bias1, bias2, wq, wk, wv, wg, wo)
    return out
